# revision 45
# baseline (speedup 1.0000x reference)
"""Distributed Trainium2 Bass kernel for multi-head attention.

Problem: x[2,2048,2048] @ qkv_w[2048,6144] -> rope(q,k) -> softmax(qk^T/sqrt(d)) @ v
         -> concat heads -> @ out_w[2048,2048].

Sharding (8 cores): core i handles batch b = i//4 and head group g = i%4
(heads 4g..4g+3).  Each core:
  1. qT,kT = (Wqk_g^T x_b^T) with rope applied         [8 x [128, 2048]]
  2. v     = x_b @ Wv_g  (natural layout)              [16 x [128, 512]]
  3. per head: S^T = k q^T (scores transposed, k on partitions),
     P = exp(S^T/sqrt(d)) (no max subtraction -- scores are O(1) here),
     out^T = v^T P (PSUM-accumulated), denominator via partition-reduce.
  4. AllGather attnT shards within the 4-core batch group -> full attnT.
  5. out[:, 512g:512(g+1)] = attnT_full^T @ out_w[:, 512g:512(g+1)].
Host: slices/transposes inputs per core, concatenates output columns.
"""

import numpy as np

from concourse import bacc, bass_isa, mybir, tile
from concourse.bass_utils import run_bass_kernel_spmd

B, N, HID = 2, 2048, 2048
H, D = 16, 128
G = 4              # head groups (tensor parallel within a batch group)
HG = H // G        # heads per group
QK_COLS = HG * D   # 512
NT = N // 128      # 16 token tiles
KT = HID // 128    # 16 hidden tiles
TC = 512           # free-dim chunk (f32 matmul max)
NTC = N // TC      # 4
OC = HID // G      # 512 output columns per core

F32 = mybir.dt.float32
F32R = mybir.dt.float32r
BF16 = mybir.dt.bfloat16
SCALE = float(1.0 / np.sqrt(D))
SWAP_MASK = [p ^ 1 for p in range(32)]  # adjacent-pair swap, uniform per 32-lane group

_NC = None
LAST_RESULT = None


def _r(ap):
    return ap  # fp32 for now; fp32r needs producer-side rounding ops


def _build(collective=True, qdt=BF16):
    nc = bacc.Bacc("TRN2", target_bir_lowering=False, debug=False, num_devices=8)

    xT = nc.dram_tensor("xT", [HID, N], F32, kind="ExternalInput")
    wqk = nc.dram_tensor("wqk", [HID, 2 * QK_COLS], F32, kind="ExternalInput")
    wv = nc.dram_tensor("wv", [HID, QK_COLS], F32, kind="ExternalInput")
    wo = nc.dram_tensor("wo", [HID, OC], F32R, kind="ExternalInput")
    cosT = nc.dram_tensor("cosT", [D, N], F32, kind="ExternalInput")
    sinT = nc.dram_tensor("sinT", [D, N], F32, kind="ExternalInput")
    rotL = nc.dram_tensor("rotL", [D, D], F32, kind="ExternalInput")
    out = nc.dram_tensor("out", [N, OC], F32, kind="ExternalOutput")

    with tile.TileContext(nc) as tc:
        with (
            tc.tile_pool(name="dram", bufs=1, space="DRAM") as dram,
            tc.tile_pool(name="pqkv", bufs=1) as pqkv,
        ):
            qkT = [pqkv.tile([128, N], qdt, name=f"qkT{m}", tag=f"qkT{m}") for m in range(2 * HG)]
            v_sb = [pqkv.tile([128, QK_COLS], qdt, name=f"v{t}", tag=f"v{t}") for t in range(NT)]

            # ---- stages 1+2: q,k (transposed, roped) and v (natural);
            # x streamed once per token chunk, reused for q/k/v matmuls ----
            with (
                tc.tile_pool(name="s1w", bufs=1) as s1w,
                tc.tile_pool(name="s1x", bufs=1) as s1x,
                tc.tile_pool(name="s1t", bufs=3) as s1t,
                tc.tile_pool(name="s1c", bufs=1) as s1c,
                tc.tile_pool(name="psqk", bufs=8, space="PSUM") as psqk,
            ):
                cos_sb = s1c.tile([D, N], F32, tag="cos")
                sin_sb = s1c.tile([D, N], F32, tag="sin")
                nc.gpsimd.dma_start(cos_sb[:], cosT[:])
                nc.gpsimd.dma_start(sin_sb[:], sinT[:])
                wqk_sb = [
                    s1w.tile([128, 2 * QK_COLS], qdt, name=f"wqk{k}", tag=f"wqk{k}")
                    for k in range(KT)
                ]
                wv_sb = [
                    s1w.tile([128, QK_COLS], qdt, name=f"wv{k}", tag=f"wv{k}")
                    for k in range(KT)
                ]
                w_loaded = [False] * KT
                wv_loaded = [False] * KT

                def load_xt(tcn, pool_only=False):
                    tsl = slice(tcn * TC, (tcn + 1) * TC)
                    xt = [
                        s1x.tile([128, TC], qdt, name=f"xt{k}", tag=f"xt{k}", bufs=2)
                        for k in range(KT)
                    ]
                    for k in range(KT):
                        xts = s1t.tile([128, TC], F32, name="xts", tag="xts")
                        deng = nc.scalar if k % 4 != 3 else nc.gpsimd
                        deng.dma_start(xts[:], xT[k * 128 : (k + 1) * 128, tsl])
                        if pool_only:
                            eng = nc.gpsimd
                        else:
                            eng = nc.gpsimd if k % 2 == 0 else nc.vector
                        eng.tensor_copy(xt[k][:], xts[:])
                    return xt

                xt = load_xt(0)
                for tcn in range(NTC):
                    tsl = slice(tcn * TC, (tcn + 1) * TC)
                    for half in range(2):
                        psums = [
                            psqk.tile([128, TC], F32, name="psqk", tag="psqk")
                            for _ in range(4)
                        ]
                        for k in range(KT):
                            if not w_loaded[k]:
                                wqs = s1t.tile([128, 2 * QK_COLS], F32, tag="wqs")
                                nc.sync.dma_start(
                                    wqs[:], wqk[k * 128 : (k + 1) * 128, :]
                                )
                                nc.vector.tensor_copy(wqk_sb[k][:], wqs[:])
                                w_loaded[k] = True
                            for mi in range(4):
                                m = half * 4 + mi
                                nc.tensor.matmul(
                                    psums[mi][:],
                                    wqk_sb[k][:, m * 128 : (m + 1) * 128],
                                    xt[k][:],
                                    start=(k == 0),
                                    stop=(k == KT - 1),
                                )
                        for mi in range(4):
                            m = half * 4 + mi
                            qsb = s1t.tile([128, TC], qdt, tag="qsb")
                            nc.scalar.activation(qsb[:], psums[mi][:], mybir.ActivationFunctionType.Copy)
                            shuf = s1t.tile([128, TC], qdt, tag="shuf")
                            nc.vector.stream_shuffle(shuf[:], qsb[:], SWAP_MASK)
                            t1 = s1t.tile([128, TC], F32, tag="t1")
                            nc.vector.tensor_tensor(
                                t1[:], qsb[:], cos_sb[:, tsl], mybir.AluOpType.mult
                            )
                            t2 = s1t.tile([128, TC], F32, tag="t2")
                            nc.vector.tensor_tensor(
                                t2[:], shuf[:], sin_sb[:, tsl], mybir.AluOpType.mult
                            )
                            nc.vector.tensor_tensor(
                                qkT[m][:, tsl], t1[:], t2[:], mybir.AluOpType.add
                            )
                    xt_next = load_xt(tcn + 1, pool_only=True) if tcn + 1 < NTC else None
                    for mtl in range(4):
                        mt = tcn * 4 + mtl
                        pv = psqk.tile([128, QK_COLS], F32, name="psv", tag="psqk")
                        for k in range(KT):
                            if not wv_loaded[k]:
                                wvs = s1t.tile([128, QK_COLS], F32, tag="wvs")
                                deng2 = nc.sync if k % 2 == 0 else nc.scalar
                                deng2.dma_start(
                                    wvs[:], wv[k * 128 : (k + 1) * 128, :]
                                )
                                nc.vector.tensor_copy(wv_sb[k][:], wvs[:])
                                wv_loaded[k] = True
                            nc.tensor.matmul(
                                pv[:],
                                xt[k][:, mtl * 128 : (mtl + 1) * 128],
                                wv_sb[k][:],
                                start=(k == 0),
                                stop=(k == KT - 1),
                            )
                        nc.vector.tensor_copy(v_sb[mt][:], pv[:])
                    xt = xt_next

            # ---- stages 3-5: attention chunked over q; AG and output
            # projection of chunk j-1 pipelined behind attention of chunk j
            CHUNKS = [(0, 512), (512, 512), (1024, 512), (1536, 512)]
            cc_in = [dram.tile([HG * 128, w], F32R, name=f"cc_in{j}", tag=f"cc_in{j}") for j, (q0, w) in enumerate(CHUNKS)]
            cc_out = [dram.tile([G * HG * 128, w], F32R, name=f"cc_out{j}", tag=f"cc_out{j}") for j, (q0, w) in enumerate(CHUNKS)]
            with (
                tc.tile_pool(name="s3p", bufs=6) as s3p,
                tc.tile_pool(name="s3c", bufs=1) as s3c,
                tc.tile_pool(name="s3a", bufs=3) as s3a,
                tc.tile_pool(name="s3d", bufs=4) as s3d,
                tc.tile_pool(name="s5w", bufs=1) as s5w,
                tc.tile_pool(name="s5at", bufs=1) as s5at,
                tc.tile_pool(name="s5o", bufs=3) as s5o,
                tc.tile_pool(name="pss", bufs=4, space="PSUM") as pss,
                tc.tile_pool(name="psd", bufs=1, space="PSUM") as psd,
                tc.tile_pool(name="pso", bufs=1, space="PSUM") as pso,
                tc.tile_pool(name="psf", bufs=2, space="PSUM") as psf,
            ):
                ones_f32 = s3c.tile([128, 1], F32, tag="ones_f32")
                nc.vector.memset(ones_f32[:], 1.0)
                ones_sb = s3c.tile([128, 1], qdt, tag="ones")
                nc.vector.tensor_copy(ones_sb[:], ones_f32[:])
                wo_sb = [s5w.tile([128, OC], F32R, name=f"wo{k}", tag=f"wo{k}") for k in range(KT)]
                for k in range(KT):
                    nc.sync.dma_start(wo_sb[k][:], wo[k * 128 : (k + 1) * 128, :])

                def attention_chunk(jq):
                    q0, w = CHUNKS[jq]
                    qsl = slice(q0, q0 + w)
                    for h in range(HG):
                        po = pso.tile([128, w], F32, name="pso", tag="pso")
                        pd = psd.tile([1, w], F32, name="psd", tag="psd")
                        for ik in range(NT):
                            ps = pss.tile([128, w], F32, name="pss", tag="pss")
                            nc.tensor.matmul(
                                ps[:],
                                qkT[HG + h][:, ik * 128 : (ik + 1) * 128],
                                qkT[h][:, qsl],
                                start=True,
                                stop=True,
                            )
                            p = s3p.tile([128, w], qdt, name="p", tag="p")
                            nc.scalar.activation(
                                p[:], ps[:], mybir.ActivationFunctionType.Exp, scale=SCALE
                            )
                            nc.tensor.matmul(
                                po[:],
                                v_sb[ik][:, h * 128 : (h + 1) * 128],
                                p[:],
                                start=(ik == 0),
                                stop=(ik == NT - 1),
                            )
                            nc.tensor.matmul(
                                pd[:],
                                ones_sb[:],
                                p[:],
                                start=(ik == 0),
                                stop=(ik == NT - 1),
                            )
                        dr = s3d.tile([1, w], F32, name="dr", tag="dr")
                        nc.vector.reciprocal(dr[:], pd[:])
                        drb = s3d.tile([128, w], F32, name="drb", tag="drb")
                        nc.gpsimd.partition_broadcast(drb[:], dr[:])
                        asb = s3a.tile([128, w], F32R, name="asb", tag="asb", bufs=4)
                        nc.vector.tensor_tensor(
                            asb[:], po[:], drb[:], mybir.AluOpType.mult
                        )
                        nc.sync.dma_start(
                            cc_in[jq][h * 128 : (h + 1) * 128, :], asb[:]
                        )
                    if collective:
                        nc.gpsimd.collective_compute(
                            "AllGather",
                            mybir.AluOpType.bypass,
                            replica_groups=[[0, 1, 2, 3], [4, 5, 6, 7]],
                            ins=[cc_in[jq].opt()],
                            outs=[cc_out[jq].opt()],
                        )
                    else:  # timeline stand-in: ~AG-equivalent local traffic
                        nc.gpsimd.dma_start(
                            cc_out[jq][: HG * 128, :], cc_in[jq][:]
                        )

                def outproj_chunk(jq):
                    atb = [s5at.tile([128, TC], F32R, name=f"at{k}", tag=f"at{k % 8}", bufs=2) for k in range(KT)]
                    for k3 in range(KT):
                        nc.scalar.dma_start(
                            atb[k3][:], cc_out[jq][k3 * 128 : (k3 + 1) * 128, :]
                        )
                    for mql in range(w // 128):
                        mq = q0 // 128 + mql
                        pf = psf.tile([128, OC], F32, name="psf", tag="psf")
                        for k3 in range(KT):
                            nc.tensor.matmul(
                                pf[:],
                                atb[k3][:, mql * 128 : (mql + 1) * 128],
                                wo_sb[k3][:],
                                start=(k3 == 0),
                                stop=(k3 == KT - 1),
                            )
                        ob = s5o.tile([128, OC], F32, name="ob", tag="ob")
                        nc.vector.tensor_copy(ob[:], pf[:])
                        nc.gpsimd.dma_start(out[mq * 128 : (mq + 1) * 128, :], ob[:])

                for jq in range(len(CHUNKS)):
                    attention_chunk(jq)
                    if jq >= 1:
                        outproj_chunk(jq - 1)
                outproj_chunk(len(CHUNKS) - 1)

    nc.compile()
    return nc


def _get_nc():
    global _NC
    if _NC is None:
        _NC = _build()
    return _NC


def _prep_in_maps(x, rope, qkv_w, out_w):
    x = np.asarray(x, np.float32)
    rope = np.asarray(rope, np.float32)
    qkv_w = np.asarray(qkv_w, np.float32)
    out_w = np.asarray(out_w, np.float32)

    freqs = rope[:, 0, :]  # [N, D]
    cosT = np.ascontiguousarray(np.repeat(freqs[:, 0::2], 2, axis=1).T)  # [D, N]
    sinT = np.repeat(freqs[:, 1::2], 2, axis=1).T.copy()
    sinT[0::2, :] *= -1.0  # rope sign folded in: rot[2i] = -q[2i+1]
    sinT = np.ascontiguousarray(sinT)
    rotL = np.zeros((D, D), np.float32)
    ii = np.arange(0, D, 2)
    rotL[ii + 1, ii] = -1.0  # rotT[2i] = -qT[2i+1]
    rotL[ii, ii + 1] = 1.0  # rotT[2i+1] = qT[2i]

    qkv3 = qkv_w.reshape(HID, 3, H, D)
    xTs = [np.ascontiguousarray(x[b].T) for b in range(B)]
    in_maps = []
    for core in range(8):
        b, g = core // G, core % G
        hs = slice(g * HG, (g + 1) * HG)
        wq = qkv3[:, 0, hs, :].reshape(HID, QK_COLS)
        wk = qkv3[:, 1, hs, :].reshape(HID, QK_COLS)
        in_maps.append(
            dict(
                xT=xTs[b],
                wqk=np.ascontiguousarray(np.concatenate([wq, wk], axis=1)),
                wv=np.ascontiguousarray(qkv3[:, 2, hs, :].reshape(HID, QK_COLS)),
                wo=np.ascontiguousarray(out_w[:, g * OC : (g + 1) * OC]),
                cosT=cosT,
                sinT=sinT,
                rotL=rotL,
            )
        )
    return in_maps


def kernel(x, rope, qkv_w, out_w):
    global LAST_RESULT
    nc = _get_nc()
    in_maps = _prep_in_maps(x, rope, qkv_w, out_w)
    res = run_bass_kernel_spmd(nc, in_maps, core_ids=list(range(8)))
    LAST_RESULT = res
    outs = [r["out"] for r in res.results]
    full = np.stack(
        [np.concatenate([outs[b * G + g] for g in range(G)], axis=1) for b in range(B)]
    )
    return full.astype(np.float32)
